# revision 1
# baseline (speedup 1.0000x reference)
"""Trainium2 Bass kernel for nn_LstmNetTest (2x LSTM + FC head).

Strategy (8 NeuronCores):
  - The dominant cost is xg1 = x_flat @ Wih1.T  ([256, 346112] x [346112, 64]).
    Shard the contraction dim K=346112 across 8 cores (43264 each); each core
    computes a partial [64, 256] gate projection with 338 accumulating
    matmuls (stationary = Wih1^T K-tile [128,64], moving = x^T K-tile [128,256]).
  - The tiny LSTM2 input projection ([5] -> [64]) is computed on every core
    with weights pre-scaled by 1/8 so the AllReduce sum is exact.
  - Both projections are rearranged into an "alt" gate tensor [32, 4*256]
    (partitions = state dims of both LSTMs, free = gate_type*256 + b*32 + t)
    and AllReduce-summed across cores.
  - Every core then runs the fused 32-step double-LSTM recurrence: per step
    4 tiny matmuls (one per gate type, both LSTMs in one [32,32] stationary
    block-diag weight) + partition-aligned elementwise ops, then the FC head
    (which also folds the h1+h2 sum via a stacked stationary). Output is
    taken from core 0.

Host side only reshapes/transposes/shards (no arithmetic on activations).
"""

import sys

for _p in ("/opt/trn_rl_repo",):
    if _p not in sys.path:
        sys.path.insert(0, _p)

import numpy as np

# Problem constants (hardcoded per contest rules)
B, S = 8, 32
H = 16
D1 = 128 * 52 * 52  # 346112
D2 = 5
G = 4 * H  # 64
M = B * S  # 256
NCORES = 8
KSH = D1 // NCORES  # 43264
KTILES = KSH // 128  # 338
CHUNK = 13  # K-tiles per DMA chunk
NCHUNK = KTILES // CHUNK  # 26

USE_F32R = True  # fp32 matmul at full PE rate (moving free dim 256 >= 256)
USE_BF16_W = False  # walrus rejects mixed f32r x bf16 matmul inputs
USE_BF16_GEMM = True  # DMA W as fp16 and cast x->fp16 on the (idle) DVE

_CACHE = {}

# gate-type order we use: a = 0:i, 1:f, 2:o, 3:g~ ; torch row blocks i,f,g,o
_TAU = [0, 1, 3, 2]


def _gate_perm():
    # rows of Wih* reordered to our (a, p) order
    return np.concatenate([np.arange(t * H, (t + 1) * H) for t in _TAU])


def _build_bass(num_devices=NCORES, phases="full"):
    import concourse.bacc as bacc
    import concourse.mybir as mybir
    import concourse.tile as tile

    F32 = mybir.dt.float32
    FIN = mybir.dt.float32r if USE_F32R else F32
    WDT = mybir.dt.float16 if USE_BF16_GEMM else (mybir.dt.bfloat16 if USE_BF16_W else FIN)
    BF16 = mybir.dt.float16
    if USE_BF16_GEMM:
        FIN = F32  # x arrives fp32, cast to bf16 on-device before the matmul
    ACT = mybir.ActivationFunctionType

    nc = bacc.Bacc(
        "TRN2",
        target_bir_lowering=False,
        debug=False,
        num_devices=num_devices,
    )

    xs_d = nc.dram_tensor("xs", [NCHUNK, 128, CHUNK * M], FIN, kind="ExternalInput")
    ws_d = nc.dram_tensor("ws", [NCHUNK, 128, CHUNK * G], WDT, kind="ExternalInput")
    lt_d = nc.dram_tensor("lt", [D2, M], F32, kind="ExternalInput")
    w2t_d = nc.dram_tensor("w2t", [D2, G], F32, kind="ExternalInput")
    # whh: [32, 4*32] — per gate type a, block-diag [k=(l',p'), m=(l,p)]
    whh_d = nc.dram_tensor("whh", [2 * H, 4 * 2 * H], F32, kind="ExternalInput")
    b1c_d = nc.dram_tensor("b1c", [G, 1], F32, kind="ExternalInput")
    b2c_d = nc.dram_tensor("b2c", [G, 1], F32, kind="ExternalInput")
    fcwt_d = nc.dram_tensor("fcwt", [2 * H, 4], F32, kind="ExternalInput")
    fcb_d = nc.dram_tensor("fcb", [4, 1], F32, kind="ExternalInput")
    eye_d = nc.dram_tensor("eye", [2 * H, 2 * H], F32, kind="ExternalInput")
    out_d = nc.dram_tensor("out", [4, M], F32, kind="ExternalOutput")

    with tile.TileContext(nc) as tc:
        with (
            tc.tile_pool(name="xp", bufs=4) as xp,
            tc.tile_pool(name="wp", bufs=3) as wp,
            tc.tile_pool(name="const", bufs=1) as cp,
            tc.tile_pool(name="state", bufs=1) as stp,
            tc.tile_pool(name="step", bufs=6) as spp,
            tc.tile_pool(name="acc", bufs=1, space="PSUM") as psp,
            tc.tile_pool(name="psg", bufs=2, space="PSUM") as psg,
            tc.tile_pool(name="dram", bufs=1, space="DRAM") as dp,
        ):
            psum1 = psp.tile([G, M], F32)
            psum2 = psp.tile([G, M], F32)

            # Big GEMM: partial xg1 = Wih1_shard @ x_shard^T  -> [64, 256]
            # (bulk DMAs own the SP ring)
            n_mm = NCHUNK * CHUNK
            for c in range(NCHUNK):
                x_t = xp.tile([128, CHUNK * M], FIN)
                w_t = wp.tile([128, CHUNK * G], WDT)
                nc.sync.dma_start(x_t[:], xs_d[c])
                nc.sync.dma_start(w_t[:], ws_d[c])
                if USE_BF16_GEMM:
                    xb_t = xp.tile([128, CHUNK * M], BF16, tag="xb")
                    nc.vector.tensor_copy(xb_t[:], x_t[:])
                    mm_x = xb_t
                else:
                    mm_x = x_t
                njs = 1 if phases == "dma" else CHUNK
                for j in range(njs):
                    idx = c * CHUNK + j
                    nc.tensor.matmul(
                        psum1[:],
                        w_t[:, j * G : (j + 1) * G],
                        mm_x[:, j * M : (j + 1) * M],
                        start=(c == 0 and j == 0),
                        stop=(idx == n_mm - 1 or (phases == "dma" and c == NCHUNK - 1)),
                    )

            # constants (issued after the bulk stream)
            lt_t = cp.tile([D2, M], F32)
            nc.sync.dma_start(lt_t[:], lt_d[:])
            w2t_t = cp.tile([D2, G], F32)
            nc.sync.dma_start(w2t_t[:], w2t_d[:])
            whh_t = cp.tile([2 * H, 4 * 2 * H], F32)
            nc.sync.dma_start(whh_t[:], whh_d[:])
            b1c_t = cp.tile([G, 1], F32)
            nc.sync.dma_start(b1c_t[:], b1c_d[:])
            b2c_t = cp.tile([G, 1], F32)
            nc.sync.dma_start(b2c_t[:], b2c_d[:])
            fcwt_t = cp.tile([2 * H, 4], F32)
            nc.sync.dma_start(fcwt_t[:], fcwt_d[:])
            fcb_t = cp.tile([4, 1], F32)
            nc.sync.dma_start(fcb_t[:], fcb_d[:])
            eye_t = cp.tile([2 * H, 2 * H], F32)
            nc.sync.dma_start(eye_t[:], eye_d[:])

            # LSTM2 input projection (weights pre-scaled by 1/NCORES)
            nc.tensor.matmul(psum2[:], w2t_t[:], lt_t[:], start=True, stop=True)

            # PSUM -> SBUF with the per-core bias share (bias/8) folded in,
            # then DMA into the "alt" DRAM layout [32, 4*256] and AllReduce.
            sb1 = stp.tile([G, M], F32)
            sb2 = stp.tile([G, M], F32)
            nc.vector.tensor_add(sb1[:], psum1[:], b1c_t[:].broadcast_to([G, M]))
            nc.vector.tensor_add(sb2[:], psum2[:], b2c_t[:].broadcast_to([G, M]))
            arin = dp.tile([2 * H, 4 * M], F32)
            arout = dp.tile([2 * H, 4 * M], F32)
            # arin element (l, p, a, n) at [l*16+p, a*256+n]; sb_l stream is
            # (a, p, n)-ordered, so view arin as [l][a, p, n]. The two DMAs go
            # to different HWDGE rings (SP vs ACT) so their ~2us fixed
            # latencies overlap.
            arin_v = arin[:].rearrange("(l p) (a n) -> l a p n", l=2, a=4)
            nc.sync.dma_start(arin_v[0], sb1[:])
            nc.scalar.dma_start(arin_v[1], sb2[:])
            if num_devices > 1:
                nc.gpsimd.collective_compute(
                    "AllReduce",
                    mybir.AluOpType.add,
                    replica_groups=[list(range(num_devices))],
                    ins=[arin[:].opt()],
                    outs=[arout[:].opt()],
                )
            else:
                nc.sync.dma_start(arout[:], arin[:])

            # xg: [32, (a=4, b=8, t=32)] — bias already folded in pre-AR
            xg = stp.tile([2 * H, 4 * M], F32)
            nc.sync.dma_start(xg[:], arout[:])

            if phases != "full":
                nc.sync.dma_start(out_d[:], xg[0:4, 0:M])

            # Fused double-LSTM recurrence.
            # State rows: h1/c1 [0:16], h2/c2 [16:32]. Gates in free dim.
            # The batch is split into NQ independent chains, interleaved per
            # step so one chain's cross-engine handoff latency hides under the
            # other chains' execution.
            if phases == "full":
                NQ = 2
                BB = B // NQ
                hs = stp.tile([2 * H, M], F32)  # free = b*32 + t
                h0 = stp.tile([2 * H, BB], F32)
                nc.any.memset(h0[:], 0.0)
                cts = []
                for q in range(NQ):
                    ctq = stp.tile([2 * H, BB], F32, tag=f"ct{q}")
                    nc.any.memset(ctq[:], 0.0)
                    cts.append(ctq)
                xg_v = xg[:].rearrange("p (a b t) -> p t a b", a=4, t=S)
                hs_v = hs[:].rearrange("p (b t) -> p t b", t=S)
                # sigmoid(x) = 0.5*tanh(x/2) + 0.5: the x/2 for the i/f/o gates
                # is pre-baked into the weights on the host, so each step needs
                # a single un-scaled Tanh over all 4 gates (Sigmoid+Tanh share
                # no ACT table; per-instruction function switches would cost a
                # 1283ns table load). The +xg add is done on the PE via an
                # identity-matmul accumulation so ACT can read PSUM directly.
                for t in range(S):
                    for q in range(NQ):
                        bsl = slice(q * BB, (q + 1) * BB)
                        ct = cts[q]
                        h_prev = h0[:] if t == 0 else hs_v[:, t - 1, bsl]
                        pg = psg.tile([2 * H, 4 * BB], F32, tag=f"pg{q}")
                        nc.tensor.matmul(
                            pg[:].rearrange("p (a b) -> p a b", a=4),
                            eye_t[:],
                            xg_v[:, t, :, bsl],
                            start=True,
                            stop=False,
                            skip_group_check=True,
                        )
                        for a in range(4):
                            nc.tensor.matmul(
                                pg[:, a * BB : (a + 1) * BB],
                                whh_t[:, a * 2 * H : (a + 1) * 2 * H],
                                h_prev,
                                start=False,
                                stop=(a == 3),
                                skip_group_check=True,
                            )
                        g = spp.tile([2 * H, 4 * BB], F32, tag=f"g{q}")
                        nc.scalar.activation(g[:], pg[:], ACT.Tanh)
                        nc.vector.tensor_scalar(
                            g[:, 0 : 3 * BB],
                            g[:, 0 : 3 * BB],
                            0.5,
                            0.5,
                            mybir.AluOpType.mult,
                            mybir.AluOpType.add,
                        )
                        t1 = spp.tile([2 * H, BB], F32, tag=f"t1_{q}")
                        t2 = spp.tile([2 * H, BB], F32, tag=f"t2_{q}")
                        nc.vector.tensor_mul(t1[:], g[:, BB : 2 * BB], ct[:])
                        nc.vector.tensor_mul(
                            t2[:], g[:, 0:BB], g[:, 3 * BB : 4 * BB]
                        )
                        nc.vector.tensor_add(ct[:], t1[:], t2[:])
                        th = spp.tile([2 * H, BB], F32, tag=f"th{q}")
                        nc.scalar.activation(th[:], ct[:], ACT.Tanh)
                        nc.vector.tensor_mul(
                            hs_v[:, t, bsl], g[:, 2 * BB : 3 * BB], th[:]
                        )

                # FC head: out^T [4, 256] = fcW @ h1 + fcW @ h2 + fcb
                pf = psp.tile([4, M], F32)
                nc.tensor.matmul(pf[:], fcwt_t[:], hs[:], start=True, stop=True)
                outt = stp.tile([4, M], F32)
                nc.vector.tensor_add(outt[:], pf[:], fcb_t[:].broadcast_to([4, M]))
                nc.sync.dma_start(out_d[:], outt[:])

    nc.compile()
    return nc


def _prep_inputs(x, l, Wih1, Whh1, bih1, bhh1, Wih2, Whh2, bih2, bhh2, fcW, fcb):
    perm = _gate_perm()
    f32 = np.float32

    xf = np.asarray(x, f32).reshape(M, D1)
    # i/f/o rows (a < 3) carry the extra 1/2 for the sigmoid-via-tanh trick
    hsc = np.repeat([0.5, 0.5, 0.5, 1.0], H)[:, None].astype(f32)  # [64, 1]
    W1p = np.asarray(Wih1, f32)[perm] * hsc  # [64, D1]

    lt = np.ascontiguousarray(np.asarray(l, f32).reshape(M, D2).T)  # [5, 256]
    w2t = np.ascontiguousarray((np.asarray(Wih2, f32)[perm] * hsc / NCORES).T)  # [5, 64]

    # whh [32, 4*32]: per gate type a: block-diag over the two LSTMs,
    # whh[:, a*32:(a+1)*32][k, m] with k = prev-h dim, m = out state dim
    W1h = np.asarray(Whh1, f32)  # [64, 16] torch order
    W2h = np.asarray(Whh2, f32)
    whh = np.zeros((2 * H, 4 * 2 * H), f32)
    for a, tau in enumerate(_TAU):
        gsc = 0.5 if a < 3 else 1.0
        blk = whh[:, a * 2 * H : (a + 1) * 2 * H]
        blk[0:H, 0:H] = W1h[tau * H : (tau + 1) * H].T * gsc  # [k', m]
        blk[H : 2 * H, H : 2 * H] = W2h[tau * H : (tau + 1) * H].T * gsc

    # per-core bias shares in psum row order (a, p), incl. the i/f/o 1/2 scale
    b1c = ((np.asarray(bih1, f32) + np.asarray(bhh1, f32))[perm] * hsc[:, 0] / NCORES)
    b2c = ((np.asarray(bih2, f32) + np.asarray(bhh2, f32))[perm] * hsc[:, 0] / NCORES)

    fcwt = np.concatenate([np.asarray(fcW, f32).T] * 2, axis=0)  # [32, 4]
    fcb_c = np.ascontiguousarray(np.asarray(fcb, f32).reshape(4, 1))

    base = dict(
        lt=lt,
        w2t=w2t,
        whh=np.ascontiguousarray(whh),
        b1c=np.ascontiguousarray(b1c.reshape(G, 1)),
        b2c=np.ascontiguousarray(b2c.reshape(G, 1)),
        fcwt=np.ascontiguousarray(fcwt),
        fcb=fcb_c,
        eye=np.eye(2 * H, dtype=f32),
    )

    in_maps = []
    for ci in range(NCORES):
        k0 = ci * KSH
        # x^T shard, chunk-interleaved: [NCHUNK, 128, CHUNK*M]
        xsh = xf[:, k0 : k0 + KSH].T  # [KSH, 256] (view)
        xs = np.ascontiguousarray(
            xsh.reshape(NCHUNK, CHUNK, 128, M).transpose(0, 2, 1, 3)
        ).reshape(NCHUNK, 128, CHUNK * M)
        wsh = W1p[:, k0 : k0 + KSH].T  # [KSH, 64]
        ws = np.ascontiguousarray(
            wsh.reshape(NCHUNK, CHUNK, 128, G).transpose(0, 2, 1, 3)
        ).reshape(NCHUNK, 128, CHUNK * G)
        if USE_BF16_GEMM:
            ws = ws.astype(np.float16)
        elif USE_BF16_W:
            import ml_dtypes

            ws = ws.astype(ml_dtypes.bfloat16)
        in_maps.append(dict(base, xs=xs, ws=ws))
    return in_maps


def _run(inputs, trace=False, trace_kwargs=None):
    from concourse.bass_utils import run_bass_kernel_spmd

    if "nc" not in _CACHE:
        _CACHE["nc"] = _build_bass()
    nc = _CACHE["nc"]

    in_maps = _prep_inputs(**inputs)
    kw = {}
    if trace:
        kw["trace"] = True
        if trace_kwargs:
            kw["trace_kwargs"] = trace_kwargs
    res = run_bass_kernel_spmd(nc, in_maps, list(range(NCORES)), **kw)
    out_t = res.results[0]["out"]  # [4, 256]
    out = np.ascontiguousarray(out_t.reshape(4, B, S).transpose(1, 2, 0))
    return out, res


def kernel(**inputs) -> np.ndarray:
    out, _ = _run(inputs, trace=False)
    return out



# revision 27
# speedup vs baseline: 1.6507x; 1.6507x over previous
"""Trainium2 Bass kernel for nn_LstmNetTest (2x LSTM + FC head).

Strategy (8 NeuronCores), v3 — t-block pipelined K-sharding with a woven
static schedule:
  - The dominant cost is the input projection xg1 = x_flat @ Wih1.T
    ([256, 346112] x [346112, 64]) which is DMA-bandwidth bound. x is
    shipped to device DRAM as fp16 (halves the 44.3 MB/core stream);
    Wih1's K-shard (5.5 MB fp16) is loaded once and stays SBUF-resident.
  - The contraction dim K=346112 is sharded across 8 cores (43264 each).
    The 32 timesteps are split into NT uneven blocks (small, big...,
    small). For each block the core re-sweeps its K-shard over just that
    block's columns (stationary reloads are cheap), producing a partial
    [64, 8*TB] gate projection that is bias-folded, packed to the "alt"
    gate layout [32, (a, t, b)] and AllReduce-summed per block.
  - The fused 32-step double-LSTM recurrence consumes blocks as their
    AllReduce lands. Because the PE executes strictly in order, the
    recurrence steps (whose matmuls wait on the previous step's h) are
    *woven* into the GEMM instruction stream at points where their
    inputs are predicted (from DMA-bandwidth estimates) to be ready, so
    blocked recurrence matmuls never starve runnable GEMM matmuls and
    the recurrence truly overlaps the x stream. Batch is split into 3
    staggered chains to hide cross-engine latency. The FC head runs per
    block into a persistent PSUM tile; one output DMA at the end.

Host side only reshapes/transposes/shards/casts (no arithmetic on
activations beyond the fp16 cast).
"""

import sys

for _p in ("/opt/trn_rl_repo",):
    if _p not in sys.path:
        sys.path.insert(0, _p)

import numpy as np

# Problem constants (hardcoded per contest rules)
B, S = 8, 32
H = 16
D1 = 128 * 52 * 52  # 346112
D2 = 5
G = 4 * H  # 64
M = B * S  # 256
NCORES = 8
KSH = D1 // NCORES  # 43264
KT = KSH // 128  # 338 k-tiles
import os as _os
KCH = [113, 113, 112]  # k-tiles per DMA chunk within a sweep
TBS = [4, 4, 4, 4, 4, 4, 4, 4]  # timesteps per block (sum 32)
if _os.environ.get("K_TBS"):
    TBS = [int(x) for x in _os.environ["K_TBS"].split(",")]
NT = len(TBS)
CHAIN_RING = _os.environ.get("K_CHAIN_RING", "act")  # pool | act
XPBUFS = int(_os.environ.get("K_XPBUFS", "4"))
CHAINPRIO = int(_os.environ.get("K_CHAINPRIO", "0"))  # 1 = high_priority chains
DIAG_SHORT_CHAIN = int(_os.environ.get("K_DIAG_SHORT_CHAIN", "0"))  # diagnostic only
PIN = float(_os.environ.get("K_PIN", "0"))  # rec-step schedule pin scale (0=off)




BSL = [(0, 4), (4, 8)]  # batch chains
NQ = len(BSL)

# schedule estimates (ns) for weaving the recurrence into the GEMM stream
BPNS = 0.345  # DMA bytes per ns (derated for overheads)
CHAIN_LAT = 9000.0  # psum-done -> xg-in-SBUF latency
STEP_EST = 1800.0  # recurrence step cadence

_CACHE = {}

# gate-type order we use: a = 0:i, 1:f, 2:o, 3:g~ ; torch row blocks i,f,g,o
_TAU = [0, 1, 3, 2]


def _gate_perm():
    return np.concatenate([np.arange(t * H, (t + 1) * H) for t in _TAU])


def _build_bass(num_devices=NCORES, phases="full"):
    import concourse.bacc as bacc
    import concourse.mybir as mybir
    import concourse.tile as tile

    F32 = mybir.dt.float32
    F16 = mybir.dt.float16
    ACT = mybir.ActivationFunctionType

    cols_l = [B * tb for tb in TBS]  # gemm columns per block
    xoff_l = np.concatenate([[0], np.cumsum([KT * c for c in cols_l])]).astype(int)
    hoff_l = np.concatenate([[0], np.cumsum(cols_l)]).astype(int)
    t0_l = np.concatenate([[0], np.cumsum(TBS)]).astype(int)
    CMAX = max(cols_l)
    XTOT = int(xoff_l[-1])  # 338*256

    nc = bacc.Bacc(
        "TRN2",
        target_bir_lowering=False,
        debug=False,
        num_devices=num_devices,
    )

    xs_d = nc.dram_tensor("xs", [128, XTOT], F16, kind="ExternalInput")
    ws_d = nc.dram_tensor("ws", [128, KT * G], F16, kind="ExternalInput")
    lt_d = nc.dram_tensor("lt", [D2, M], F32, kind="ExternalInput")
    w2t_d = nc.dram_tensor("w2t", [D2, G], F32, kind="ExternalInput")
    # whh: [32, 4*32] — per gate type a, block-diag [k=(l',p'), m=(l,p)]
    whh_d = nc.dram_tensor("whh", [2 * H, 4 * 2 * H], F32, kind="ExternalInput")
    b1c_d = nc.dram_tensor("b1c", [G, 1], F32, kind="ExternalInput")
    b2c_d = nc.dram_tensor("b2c", [G, 1], F32, kind="ExternalInput")
    fcwt_d = nc.dram_tensor("fcwt", [2 * H, 4], F32, kind="ExternalInput")
    fcb_d = nc.dram_tensor("fcb", [4, 1], F32, kind="ExternalInput")
    eye_d = nc.dram_tensor("eye", [2 * H, 2 * H], F32, kind="ExternalInput")
    out_d = nc.dram_tensor("out", [4, M], F32, kind="ExternalOutput")

    with tile.TileContext(nc) as tc:
        with (
            tc.tile_pool(name="wres", bufs=1) as wrp,
            tc.tile_pool(name="xp", bufs=XPBUFS) as xp,
            tc.tile_pool(name="const", bufs=1) as cp,
            tc.tile_pool(name="state", bufs=1) as stp,
            tc.tile_pool(name="sbb", bufs=2) as sbp,
            tc.tile_pool(name="xgp", bufs=3) as xgp,
            tc.tile_pool(name="step", bufs=4) as spp,
            tc.tile_pool(name="acc", bufs=2, space="PSUM") as psp,
            tc.tile_pool(name="psgA", bufs=2, space="PSUM") as psgA,
            tc.tile_pool(name="psgB", bufs=1, space="PSUM") as psgB,
            tc.tile_pool(name="pfc", bufs=1, space="PSUM") as pfp,
            tc.tile_pool(name="dram", bufs=1, space="DRAM") as dp,
        ):
            # constants (scalar ring, issued first)
            lt_t = cp.tile([D2, M], F32, tag="lt")
            nc.scalar.dma_start(lt_t[:], lt_d[:])
            w2t_t = cp.tile([D2, G], F32, tag="w2t")
            nc.scalar.dma_start(w2t_t[:], w2t_d[:])
            whh_t = cp.tile([2 * H, 4 * 2 * H], F32, tag="whh")
            nc.scalar.dma_start(whh_t[:], whh_d[:])
            b1c_t = cp.tile([G, 1], F32, tag="b1c")
            nc.scalar.dma_start(b1c_t[:], b1c_d[:])
            b2c_t = cp.tile([G, 1], F32, tag="b2c")
            nc.scalar.dma_start(b2c_t[:], b2c_d[:])
            fcwt_t = cp.tile([2 * H, 4], F32, tag="fcwt")
            nc.scalar.dma_start(fcwt_t[:], fcwt_d[:])
            fcb_t = cp.tile([4, 1], F32, tag="fcb")
            nc.scalar.dma_start(fcb_t[:], fcb_d[:])
            eye_t = cp.tile([2 * H, 2 * H], F32, tag="eye")
            nc.scalar.dma_start(eye_t[:], eye_d[:])

            # LSTM2 input projection for all timesteps (weights and biases
            # pre-scaled by 1/NCORES so the per-block AllReduce is exact),
            # moved to SBUF once with the bias folded in.
            psum2 = psgB.tile([G, M], F32, tag="p2")
            nc.tensor.matmul(psum2[:], w2t_t[:], lt_t[:], start=True, stop=True)
            sb2 = stp.tile([G, M], F32, tag="sb2")
            nc.vector.tensor_add(sb2[:], psum2[:], b2c_t[:].broadcast_to([G, M]))

            # Whole Wih1 K-shard resident in SBUF, chunks aligned with x
            wres = wrp.tile([128, KT * G], F16, tag="wres")

            # recurrence state
            hs = stp.tile([2 * H, M], F32, tag="hs")  # free = (nb, t, b)
            h0 = stp.tile([2 * H, B], F32, tag="h0")
            nc.any.memset(h0[:], 0.0)
            cts = []
            for q in range(NQ):
                bb = BSL[q][1] - BSL[q][0]
                ctq = stp.tile([2 * H, bb], F32, tag=f"ct{q}")
                nc.any.memset(ctq[:], 0.0)
                cts.append(ctq)
            psum_fc = pfp.tile([4, M], F32, tag="fc")

            # per-block DRAM staging tiles for the AllReduce (collectives
            # need contiguous buffers). The LSTM2 halves are all written
            # up-front, right after sb2 is ready.
            arins, arouts = [], []
            for nb in range(NT):
                cols = cols_l[nb]
                arin = dp.tile([2 * H, 4 * cols], F32, tag=f"arin{nb}")
                arout = dp.tile([2 * H, 4 * cols], F32, tag=f"arout{nb}")
                arins.append(arin)
                arouts.append(arout)
                o2 = int(hoff_l[nb])
                nc.scalar.dma_start(
                    arin[:].rearrange("(l p) (a n) -> l a p n", l=2, a=4)[1],
                    sb2[:, o2 : o2 + cols],
                )

            def emit_chain(nb, psum1):
                cols = cols_l[nb]
                arin, arout = arins[nb], arouts[nb]
                # LSTM1 partial gates + bias share, PSUM -> SBUF
                sb1 = sbp.tile([G, CMAX], F32, tag="sb1")
                nc.vector.tensor_add(
                    sb1[:, :cols], psum1[:, :cols], b1c_t[:].broadcast_to([G, cols])
                )
                nc.scalar.dma_start(
                    arin[:].rearrange("(l p) (a n) -> l a p n", l=2, a=4)[0],
                    sb1[:, :cols],
                )
                if num_devices > 1:
                    nc.gpsimd.collective_compute(
                        "AllReduce",
                        mybir.AluOpType.add,
                        replica_groups=[list(range(num_devices))],
                        ins=[arin[:].opt()],
                        outs=[arout[:].opt()],
                    )
                else:
                    nc.gpsimd.dma_start(arout[:], arin[:])
                xg = xgp.tile([2 * H, 4 * CMAX], F32, tag="xg")
                nc.gpsimd.dma_start(xg[:, : 4 * cols], arout[:])
                return xg

            def emit_step(nb, dt, xg):
                tb = TBS[nb]
                ho = int(hoff_l[nb])
                gt = int(t0_l[nb]) + dt
                xg_v = xg[:, : 4 * cols_l[nb]].rearrange(
                    "p (a t b) -> p t a b", t=tb, a=4
                )
                for q in range(NQ):
                    b0, b1 = BSL[q]
                    bb = b1 - b0
                    ct = cts[q]
                    if gt == 0:
                        h_prev = h0[:, b0:b1]
                    elif dt == 0:
                        po = int(hoff_l[nb - 1]) + (TBS[nb - 1] - 1) * B
                        h_prev = hs[:, po + b0 : po + b1]
                    else:
                        po = ho + (dt - 1) * B
                        h_prev = hs[:, po + b0 : po + b1]
                    pool = psgA
                    pg = pool.tile([2 * H, 4 * bb], F32, tag=f"pg{q}")
                    nc.tensor.matmul(
                        pg[:].rearrange("p (a b) -> p a b", a=4),
                        eye_t[:],
                        xg_v[:, dt, :, b0:b1],
                        start=True,
                        stop=False,
                        skip_group_check=True,
                    )
                    for a in range(4):
                        nc.tensor.matmul(
                            pg[:, a * bb : (a + 1) * bb],
                            whh_t[:, a * 2 * H : (a + 1) * 2 * H],
                            h_prev,
                            start=False,
                            stop=(a == 3),
                            skip_group_check=True,
                        )
                    # One Tanh over all 4 gates: g = tanh(pg/2). For i,f,o
                    # this is the sigmoid half-tanh (sigma = (g+1)/2); the
                    # candidate's weights are host-doubled so tanh(z) comes
                    # out exact. Cell state is tracked as S = 2c; h as 2h
                    # (Whh and fcW host-halved), which turns the whole cell
                    # update into fused scalar_tensor_tensor ops.
                    g = spp.tile([2 * H, 4 * bb], F32, tag=f"g{q}")
                    nc.scalar.activation(g[:], pg[:], ACT.Tanh, scale=0.5)
                    t1 = spp.tile([2 * H, bb], F32, tag=f"t1_{q}")
                    t2 = spp.tile([2 * H, bb], F32, tag=f"t2_{q}")
                    # t1 = (g_f + 1) * S_prev ; t2 = (g_i + 1) * g~
                    nc.vector.scalar_tensor_tensor(
                        t1[:], g[:, bb : 2 * bb], 1.0, ct[:],
                        mybir.AluOpType.add, mybir.AluOpType.mult,
                    )
                    nc.vector.scalar_tensor_tensor(
                        t2[:], g[:, 0:bb], 1.0, g[:, 3 * bb : 4 * bb],
                        mybir.AluOpType.add, mybir.AluOpType.mult,
                    )
                    # S_new = t1/2 + t2  (= 2*c_new)
                    nc.vector.scalar_tensor_tensor(
                        ct[:], t1[:], 0.5, t2[:],
                        mybir.AluOpType.mult, mybir.AluOpType.add,
                    )
                    th = spp.tile([2 * H, bb], F32, tag=f"th{q}")
                    nc.scalar.activation(th[:], ct[:], ACT.Tanh, scale=0.5)
                    # hs = (g_o + 1) * tanh(c) = 2h
                    nc.vector.scalar_tensor_tensor(
                        hs[:, ho + dt * B + b0 : ho + dt * B + b1],
                        g[:, 2 * bb : 3 * bb], 1.0, th[:],
                        mybir.AluOpType.add, mybir.AluOpType.mult,
                    )
                if dt == tb - 1:
                    # FC head for this block: out^T slice = [fcW;fcW] @ hs_blk
                    nc.tensor.matmul(
                        psum_fc[:, ho : ho + cols_l[nb]],
                        fcwt_t[:],
                        hs[:, ho : ho + cols_l[nb]],
                        start=True,
                        stop=True,
                        skip_group_check=True,
                    )

            # ---- pinned static schedule ----
            # Every stage is annotated with a virtual timestamp (ns) via
            # tile_wait_until so the Tile scheduler reproduces the intended
            # pipeline: GEMM sweeps paced by the DMA stream, the per-block
            # AllReduce chain right after its sweep, and each recurrence
            # step at its predicted data-ready time. Without the pins the
            # scheduler's greedy order makes the counting semaphores encode
            # false cross-block dependencies.
            est = 2000.0  # consts
            r = 0.0
            for nb in range(NT):
                cols = cols_l[nb]
                psum1 = psp.tile([G, CMAX], F32, tag="acc")
                koff = 0
                for c, ktc in enumerate(KCH):
                    if nb == 0:
                        wch = wres[:, koff * G : (koff + ktc) * G]
                        nc.sync.dma_start(wch, ws_d[:, koff * G : (koff + ktc) * G])
                        est += ktc * G * 128 * 2 / BPNS
                    x_t = xp.tile([128, max(KCH) * CMAX], F16, tag="x")
                    xo = int(xoff_l[nb]) + koff * cols
                    nc.sync.dma_start(
                        x_t[:, : ktc * cols], xs_d[:, xo : xo + ktc * cols]
                    )
                    est += ktc * cols * 128 * 2 / BPNS + 300.0
                    for j in range(ktc):
                        jg = koff + j
                        nc.tensor.matmul(
                            psum1[:, :cols],
                            wres[:, jg * G : (jg + 1) * G],
                            x_t[:, j * cols : (j + 1) * cols],
                            start=(jg == 0),
                            stop=(jg == KT - 1),
                        )
                    koff += ktc
                if CHAINPRIO:
                    with tc.high_priority():
                        xg = emit_chain(nb, psum1)
                else:
                    xg = emit_chain(nb, psum1)
                xg_ready = est + CHAIN_LAT
                for dt in range(TBS[nb]):
                    r = max(r + STEP_EST, xg_ready)
                    if PIN > 0:
                        with tc.tile_wait_until(PIN * r / 1e6):
                            emit_step(nb, dt, xg)
                    else:
                        emit_step(nb, dt, xg)

            outt = stp.tile([4, M], F32, tag="outt")
            nc.vector.tensor_add(outt[:], psum_fc[:], fcb_t[:].broadcast_to([4, M]))
            nc.sync.dma_start(out_d[:], outt[:])

    nc.compile()
    return nc


def _prep_inputs(x, l, Wih1, Whh1, bih1, bhh1, Wih2, Whh2, bih2, bhh2, fcW, fcb):
    perm = _gate_perm()
    f32 = np.float32
    f16 = np.float16

    cols_l = [B * tb for tb in TBS]
    t0_l = np.concatenate([[0], np.cumsum(TBS)]).astype(int)

    xf = np.asarray(x, f32).reshape(M, D1)
    # candidate-gate (a=3) input rows doubled: tanh((2z)/2) == tanh(z)
    gsc = np.repeat([1.0, 1.0, 1.0, 2.0], H)[:, None].astype(f32)  # [64, 1]
    W1p = np.asarray(Wih1, f32)[perm] * gsc  # [64, D1]

    # columns in block-major (nb, dt, b) order everywhere
    def msel(nb):
        return [
            b * S + int(t0_l[nb]) + dt for dt in range(TBS[nb]) for b in range(B)
        ]

    lcols = np.concatenate([msel(nb) for nb in range(NT)]).astype(int)
    lt = np.ascontiguousarray(np.asarray(l, f32).reshape(M, D2).T[:, lcols])
    w2t = np.ascontiguousarray((np.asarray(Wih2, f32)[perm] * gsc / NCORES).T)

    # whh [32, 4*32]: per gate type a: block-diag over the two LSTMs
    W1h = np.asarray(Whh1, f32)
    W2h = np.asarray(Whh2, f32)
    whh = np.zeros((2 * H, 4 * 2 * H), f32)
    # h is stored as 2h -> halve Whh; candidate gate rows doubled again
    for a, tau in enumerate(_TAU):
        ws = 1.0 if a == 3 else 0.5  # 2 * 0.5 for a=3
        blk = whh[:, a * 2 * H : (a + 1) * 2 * H]
        blk[0:H, 0:H] = W1h[tau * H : (tau + 1) * H].T * ws
        blk[H : 2 * H, H : 2 * H] = W2h[tau * H : (tau + 1) * H].T * ws

    b1c = (np.asarray(bih1, f32) + np.asarray(bhh1, f32))[perm] * gsc[:, 0] / NCORES
    b2c = (np.asarray(bih2, f32) + np.asarray(bhh2, f32))[perm] * gsc[:, 0] / NCORES

    # hs holds 2h -> fold the 1/2 into the FC weights
    fcwt = np.concatenate([np.asarray(fcW, f32).T * 0.5] * 2, axis=0)  # [32, 4]
    fcb_c = np.ascontiguousarray(np.asarray(fcb, f32).reshape(4, 1))

    base = dict(
        lt=lt,
        w2t=w2t,
        whh=np.ascontiguousarray(whh),
        b1c=np.ascontiguousarray(b1c.reshape(G, 1)),
        b2c=np.ascontiguousarray(b2c.reshape(G, 1)),
        fcwt=np.ascontiguousarray(fcwt),
        fcb=fcb_c,
        eye=np.eye(2 * H, dtype=f32),
    )

    in_maps = []
    for ci in range(NCORES):
        k0 = ci * KSH
        A = xf[:, k0 : k0 + KSH].astype(f16)  # [256, KSH] fp16
        parts = []
        for nb in range(NT):
            # [cols, KT, 128] -> [128, KT, cols]
            a_nb = A[msel(nb)].reshape(cols_l[nb], KT, 128).transpose(2, 1, 0)
            parts.append(a_nb.reshape(128, KT * cols_l[nb]))
        xs = np.ascontiguousarray(np.concatenate(parts, axis=1))
        wsh = W1p[:, k0 : k0 + KSH].T.astype(f16)  # [KSH, 64]
        ws = np.ascontiguousarray(
            wsh.reshape(KT, 128, G).transpose(1, 0, 2)
        ).reshape(128, KT * G)
        in_maps.append(dict(base, xs=xs, ws=ws))
    return in_maps


def _unshuffle_out(out_t):
    # out_t [4, 256] columns are (nb, dt, b); return [B, S, 4]
    t0_l = np.concatenate([[0], np.cumsum(TBS)]).astype(int)
    hoff_l = np.concatenate([[0], np.cumsum([B * tb for tb in TBS])]).astype(int)
    out = np.empty((B, S, 4), np.float32)
    for nb in range(NT):
        blk = out_t[:, hoff_l[nb] : hoff_l[nb + 1]].reshape(4, TBS[nb], B)
        out[:, t0_l[nb] : t0_l[nb + 1], :] = blk.transpose(2, 1, 0)
    return out


def _run(inputs, trace=False, trace_kwargs=None):
    from concourse.bass_utils import run_bass_kernel_spmd

    if "nc" not in _CACHE:
        _CACHE["nc"] = _build_bass()
    nc = _CACHE["nc"]

    in_maps = _prep_inputs(**inputs)
    kw = {}
    if trace:
        kw["trace"] = True
        if trace_kwargs:
            kw["trace_kwargs"] = trace_kwargs
    res = run_bass_kernel_spmd(nc, in_maps, list(range(NCORES)), **kw)
    out_t = res.results[0]["out"]  # [4, 256]
    return _unshuffle_out(np.asarray(out_t)), res


def kernel(**inputs) -> np.ndarray:
    out, _ = _run(inputs, trace=False)
    return out
